# revision 1
# baseline (speedup 1.0000x reference)
"""Causal multi-head self-attention with RoPE on 8 TRN2 NeuronCores.

Sharding: data-parallel over batch (4) x tensor-parallel over heads (16 -> 2
groups of 8).  Core c handles batch c//2, head group c%2.  Each core computes
its 8 heads' attention and a partial O-projection (512 of the 1024 contraction
dims); the host sums the two partials per batch element.

Emission strategy: attention is ACT(exp)-paced (+105ns/tile over PE), so
proj/oproj matmuls are queued as "filler units" pulled into the attention
tile loop via a PE-idle credit counter, keeping PE saturated end-to-end.
"""

import os
import sys

import numpy as np

if "/opt/trn_rl_repo" not in sys.path:
    sys.path.insert(0, "/opt/trn_rl_repo")

D_MODEL = 1024
NUM_HEADS = 16
THETA = 10000.0
B, S = 4, 2048
DK = 64
HALF = DK // 2
P = 128
N_CORES = 8
HPC = 8                 # heads per core
DOUT = HPC * DK         # 512 per-core projected dims
KT = D_MODEL // P       # 8 contraction tiles
NSEQ = S // P           # 16 seq tiles of 128
SCALE = 1.0 / np.sqrt(DK)

_CACHE = {}


def _build():
    """Build + compile the per-core Bass module (same program on all cores)."""
    import concourse.bass as bass
    import concourse.bacc as bacc
    import concourse.tile as tile
    import concourse.mybir as mybir
    from contextlib import ExitStack
    from collections import deque

    f32 = mybir.dt.float32
    bf16 = mybir.dt.bfloat16
    Exp = mybir.ActivationFunctionType.Exp

    nc = bacc.Bacc("TRN2", target_bir_lowering=False, debug=False,
                   enable_asserts=False, num_devices=N_CORES)

    # all inputs pre-tiled on host into SBUF layout (partition-contiguous)
    xt = nc.dram_tensor("xt", [P, NSEQ * D_MODEL], bf16, kind="ExternalInput")
    wq = nc.dram_tensor("wq", [P, KT * DOUT], bf16, kind="ExternalInput")
    wk = nc.dram_tensor("wk", [P, KT * DOUT], bf16, kind="ExternalInput")
    wv = nc.dram_tensor("wv", [P, KT * DOUT], bf16, kind="ExternalInput")
    wo = nc.dram_tensor("wo", [P, (DOUT // P) * D_MODEL], bf16,
                        kind="ExternalInput")
    cosn = nc.dram_tensor("cosn", [P, NSEQ * DK], bf16, kind="ExternalInput")
    sinn = nc.dram_tensor("sinn", [P, NSEQ * DK], bf16, kind="ExternalInput")
    maskt = nc.dram_tensor("maskt", [P, P], bf16, kind="ExternalInput")
    ident = nc.dram_tensor("ident", [P, P], bf16, kind="ExternalInput")
    # output in tiled layout too: [p, m*1024 + c] = partial_out[m*128+p, c]
    out = nc.dram_tensor("out", [P, NSEQ * D_MODEL], bf16,
                         kind="ExternalOutput")

    def rep8(ap):
        # replicate a [128, 64] tile 8x along free dim -> logical [128, 512]
        return bass.AP(tensor=ap.tensor, offset=ap.offset,
                       ap=[ap.ap[0], [0, HPC], [1, DK]])

    def pairswap(ap):
        # free-dim pair swap of a [128, 512] tile: (0,1,2,3,..)->(1,0,3,2,..)
        return bass.AP(tensor=ap.tensor, offset=ap.offset + 1,
                       ap=[ap.ap[0], [2, 256], [-1, 2]])

    with tile.TileContext(nc) as tc, ExitStack() as top:
        persist = top.enter_context(tc.tile_pool(name="persist", bufs=1))
        # psum pools (8 banks): proj/oproj 2, transpose 2, scores 2, AV 2
        mm_ps = top.enter_context(tc.tile_pool(name="mm_ps", bufs=3, space="PSUM"))
        tr_ps = top.enter_context(tc.tile_pool(name="tr_ps", bufs=1, space="PSUM"))
        sc_ps = top.enter_context(tc.tile_pool(name="sc_ps", bufs=2, space="PSUM"))
        av_ps = top.enter_context(tc.tile_pool(name="av_ps", bufs=2, space="PSUM"))
        ropet = top.enter_context(tc.tile_pool(name="ropet", bufs=2))
        natp = top.enter_context(tc.tile_pool(name="natp", bufs=4))
        rcpp = top.enter_context(tc.tile_pool(name="rcpp", bufs=2))
        rmatp = top.enter_context(tc.tile_pool(name="rmatp", bufs=2))
        ostg = top.enter_context(tc.tile_pool(name="ostg", bufs=3))

        # ---- persistent SBUF arrays ----
        x_sb = persist.tile([P, NSEQ * D_MODEL], bf16, tag="x", name="x")
        w_sb = {nm: persist.tile([P, KT * DOUT], bf16, tag=nm, name=nm)
                for nm in ("wq", "wk", "wv")}
        wo_sb = persist.tile([P, (DOUT // P) * D_MODEL], bf16, tag="wo",
                             name="wo")
        cos_sb = persist.tile([P, NSEQ * DK], bf16, tag="cos", name="cos")
        sin_sb = persist.tile([P, NSEQ * DK], bf16, tag="sin", name="sin")
        mask_sb = persist.tile([P, P], bf16, tag="mask", name="mask")
        id_sb = persist.tile([P, P], bf16, tag="ident", name="ident")
        pt_sb = [persist.tile([P, 512], bf16, tag=f"pt{i}", name=f"pt{i}")
                 for i in range(4)]
        qt_sb = persist.tile([P, 4 * S], bf16, tag="qt", name="qt")
        kt_sb = persist.tile([P, 4 * S], bf16, tag="kt", name="kt")
        ot_sb = persist.tile([P, 4 * S], bf16, tag="ot", name="ot")
        v_sb = [persist.tile([P, HPC * (DK + 1)], bf16, tag=f"v{t}",
                             name=f"v{t}") for t in range(NSEQ)]

        def xs(m, k):
            """x_sb slice for seq tile m, contraction tile k: [128, 128]."""
            c = m * D_MODEL + k * P
            return x_sb[:, c:c + P]

        # ---- input DMAs: contiguous per-partition blocks, first-use order --
        def dma_x_chunk(g):
            c = g * 4 * D_MODEL
            nc.sync.dma_start(out=x_sb[:, c:c + 4 * D_MODEL],
                              in_=xt[:, c:c + 4 * D_MODEL])

        # split first chunk: x(m0) + wq lower half unblock the first matmuls
        nc.sync.dma_start(out=x_sb[:, 0:D_MODEL], in_=xt[:, 0:D_MODEL])
        nc.sync.dma_start(out=w_sb["wq"][:, 0:4 * DOUT],
                          in_=wq[:, 0:4 * DOUT])
        nc.sync.dma_start(out=x_sb[:, D_MODEL:2 * D_MODEL],
                          in_=xt[:, D_MODEL:2 * D_MODEL])
        nc.sync.dma_start(out=w_sb["wq"][:, 4 * DOUT:],
                          in_=wq[:, 4 * DOUT:])
        nc.sync.dma_start(out=cos_sb, in_=cosn[:, :])
        nc.sync.dma_start(out=sin_sb, in_=sinn[:, :])
        nc.sync.dma_start(out=x_sb[:, 2 * D_MODEL:4 * D_MODEL],
                          in_=xt[:, 2 * D_MODEL:4 * D_MODEL])
        nc.sync.dma_start(out=id_sb, in_=ident[:, :])
        nc.sync.dma_start(out=w_sb["wk"], in_=wk[:, :])
        nc.sync.dma_start(out=w_sb["wv"], in_=wv[:, :])
        nc.sync.dma_start(out=mask_sb, in_=maskt[:, :])
        dma_x_chunk(1)
        dma_x_chunk(2)
        dma_x_chunk(3)
        nc.sync.dma_start(out=wo_sb, in_=wo[:, :])

        def drain_ap(dst, m):
            # [128, 512] trt -> 4 dout-blocks of 128 cols at seq tile m
            return bass.AP(tensor=dst.tensor, offset=dst.offset + m * P,
                           ap=[dst.ap[0], [S, 4], [1, P]])

        def qkslice(src, db, po, c0, c1):
            # rows po..po+DK of dout-block db, seq cols [c0, c1)
            return src[po:po + DK, db * S + c0:db * S + c1]

        # ---- filler unit queue ----------------------------------------
        # each unit: (pe_cost_ns, emit_fn, kind, gidx). attention pulls
        # units whenever accumulated PE-idle credit covers the next cost.
        units = deque()

        def push_proj_group(g):
            for nm in ("wq", "wk", "wv"):
                pend_tr = []     # lag-1: m's transposes emitted after m+1 mms
                for m in range(4 * g, 4 * g + 4):
                    box = {}
                    for k in range(KT):
                        def u_mm(box=box, nm=nm, m=m, k=k):
                            if k == 0:
                                box["ps"] = mm_ps.tile([P, DOUT], f32,
                                                       tag="mm", name="mm")
                            nc.tensor.matmul(
                                box["ps"], xs(m, k),
                                w_sb[nm][:, k * DOUT:(k + 1) * DOUT],
                                start=(k == 0), stop=(k == KT - 1))
                        units.append((266, u_mm, "proj", g))
                        if k == 1 and pend_tr:
                            units.append(pend_tr.pop())
                    if nm == "wv":
                        def u_v(box=box, m=m):
                            vt = v_sb[m]
                            ones_ap = bass.AP(
                                tensor=vt.tensor, offset=vt.offset + DK,
                                ap=[vt.ap[0], [DK + 1, HPC]])
                            nc.gpsimd.memset(ones_ap, 1.0)
                            vcols = bass.AP(
                                tensor=vt.tensor, offset=vt.offset,
                                ap=[vt.ap[0], [DK + 1, HPC], [1, DK]])
                            nc.scalar.copy(vcols, box["ps"])
                        units.append((0, u_v, "proj", g))
                    else:
                        dst = qt_sb if nm == "wq" else kt_sb

                        def u_rope(box=box, m=m):
                            # bf16 rope: stage psum->bf16 once, then all
                            # muls/add run in the DVE 4x perf mode
                            ps = box["ps"]
                            nat0 = ropet.tile([P, DOUT], bf16, tag="rt0",
                                              name="rt0")
                            nc.vector.tensor_copy(nat0, ps)
                            t1 = ropet.tile([P, DOUT], bf16, tag="rt1",
                                            name="rt1")
                            t2 = ropet.tile([P, DOUT], bf16, tag="rt2",
                                            name="rt2")
                            cosm = cos_sb[:, m * DK:(m + 1) * DK]
                            sinm = sin_sb[:, m * DK:(m + 1) * DK]
                            nc.vector.tensor_mul(t1, nat0, rep8(cosm))
                            nc.vector.tensor_mul(t2, pairswap(nat0),
                                                 rep8(sinm))
                            nat = natp.tile([P, DOUT], bf16, tag="nat",
                                            name="nat")
                            nc.vector.tensor_add(nat, t1, t2)
                            box["nat"] = nat
                        units.append((0, u_rope, "proj", g))

                        def u_tr(box=box, m=m, dst=dst):
                            nat = box["nat"]
                            trt = tr_ps.tile([P, 512], bf16, tag="tr",
                                             name="trt")
                            for d in range(4):
                                nc.tensor.transpose(
                                    trt[:, d * P:(d + 1) * P],
                                    nat[:, d * P:(d + 1) * P], id_sb)
                            nc.vector.tensor_copy(drain_ap(dst, m), trt)
                        pend_tr.append((430, u_tr, "proj", g))
                if pend_tr:
                    units.append(pend_tr.pop())

        def push_oproj_group(g):
            for m in range(4 * g, 4 * g + 4):
                box = {}
                for nb in range(2):
                    for k in range(4):
                        def u_mm(box=box, m=m, nb=nb, k=k):
                            if k == 0:
                                if nb == 0:
                                    box["og"] = ostg.tile(
                                        [P, D_MODEL], bf16, tag="og",
                                        name="og")
                                box["ps"] = mm_ps.tile([P, 512], f32,
                                                       tag="mm", name="mm")
                            nc.tensor.matmul(
                                box["ps"],
                                ot_sb[:, k * S + m * P:k * S + (m + 1) * P],
                                wo_sb[:, k * D_MODEL + nb * 512:
                                      k * D_MODEL + (nb + 1) * 512],
                                start=(k == 0), stop=(k == 3))
                        units.append((266, u_mm, "oproj", g))

                    def u_og(box=box, m=m, nb=nb):
                        nc.vector.tensor_copy(
                            box["og"][:, nb * 512:(nb + 1) * 512], box["ps"])
                        c = m * D_MODEL + nb * 512
                        nc.sync.dma_start(
                            out=out[:, c:c + 512],
                            in_=box["og"][:, nb * 512:(nb + 1) * 512])
                    units.append((0, u_og, "oproj", g))

        def flush_units(pred):
            """Emit queued units until none matching pred remain."""
            while any(pred(u) for u in units):
                cost, fn, kind, gi = units.popleft()
                fn()

        credit = [0.0]

        def pull_units(gcur):
            while units:
                cost, fn, kind, gi = units[0]
                if kind == "oproj" and gi >= gcur:
                    break           # oproj(g) only after attn(g) complete
                if cost > credit[0]:
                    break
                units.popleft()
                fn()
                credit[0] -= cost

        # ---- attention ------------------------------------------------
        def attn_group(g):
            """Attention for query block g (512 cols) over all heads.

            Diagonal kv-tiles (t in [4g, 4g+4)) are trimmed to q-cols >=
            128*(t-4g) in scores, exp, and AV; a 128-wide lower-tri mask-mul
            zeroes the intra-tile upper triangle.  Stale pt prefixes are
            never read.
            """
            nt = 4 * g + 4
            for h in range(HPC):
                db, po = h // 2, (h % 2) * DK
                av = av_ps.tile([DK + 1, 512], f32, tag="av", name="av")
                for t in range(nt):
                    v = t - 4 * g
                    c0 = 128 * v if v >= 0 else 0
                    w = 512 - c0
                    sc = sc_ps.tile([P, 512], f32, tag="sc", name="sc")
                    nc.tensor.matmul(
                        sc[:, c0:],
                        qkslice(kt_sb, db, po, t * P, (t + 1) * P),
                        qkslice(qt_sb, db, po, g * 512 + c0, (g + 1) * 512),
                        start=True, stop=True)
                    pt = pt_sb[t % 4]
                    nc.scalar.activation(pt[:, c0:], sc[:, c0:], Exp)
                    if v >= 0:
                        nc.vector.tensor_mul(pt[:, c0:c0 + P],
                                             pt[:, c0:c0 + P], mask_sb)
                    nc.tensor.matmul(
                        av[:, c0:],
                        v_sb[t][:, h * (DK + 1):(h + 1) * (DK + 1)],
                        pt[:, c0:], start=(t == 0), stop=(t == nt - 1))
                    # ACT-PE imbalance for this tile (~185ns, width-free)
                    credit[0] = min(credit[0] + 185.0, 2200.0)
                    pull_units(g)
                rcp = rcpp.tile([1, 512], f32, tag="rcp", name="rcp")
                nc.vector.reciprocal(rcp, av[DK:DK + 1, :])
                rmat = rmatp.tile([DK, 512], f32, tag="rmat", name="rmat")
                nc.gpsimd.partition_broadcast(rmat, rcp, channels=DK)
                cols = slice(db * S + g * 512, db * S + (g + 1) * 512)
                nc.vector.tensor_mul(ot_sb[po:po + DK, cols],
                                     av[0:DK, :], rmat)

        # ---- schedule -------------------------------------------------
        push_proj_group(0)
        flush_units(lambda u: u[2] == "proj" and u[3] == 0)
        for g in range(4):
            if g + 1 < 4:
                push_proj_group(g + 1)
            attn_group(g)
            push_oproj_group(g)
            # attn(g+1) needs proj(g+1) complete: force-drain leftovers
            if g + 1 < 4:
                flush_units(lambda u, gg=g + 1: u[2] == "proj"
                            and u[3] == gg)
        flush_units(lambda u: True)

    nc.compile()
    return nc


def _get_nc():
    if "nc" not in _CACHE:
        _CACHE["nc"] = _build()
    return _CACHE["nc"]


def _tile_pm(a):
    """[S, C] -> [128, (S//128)*C] : row m*128+p -> partition p, col m*C+c."""
    Sv, C = a.shape
    return np.ascontiguousarray(
        a.reshape(Sv // P, P, C).transpose(1, 0, 2).reshape(P, -1))


def _prep_core_inputs(q_proj_weight, k_proj_weight, v_proj_weight,
                      o_proj_weight, in_features, token_positions):
    """Host-side sharding: returns the list of 8 per-core input dicts."""
    import ml_dtypes
    bf = ml_dtypes.bfloat16

    x = np.asarray(in_features, np.float32)
    wqf = np.asarray(q_proj_weight, np.float32)
    wkf = np.asarray(k_proj_weight, np.float32)
    wvf = np.asarray(v_proj_weight, np.float32)
    wof = np.asarray(o_proj_weight, np.float32)
    tp = np.asarray(token_positions).astype(np.float64)

    inv = 1.0 / (THETA ** (np.arange(HALF, dtype=np.float64) / HALF))
    fr = tp[:, None] * inv[None, :]                       # [S, 32]
    cosn = np.repeat(np.cos(fr), 2, axis=1).astype(np.float32)  # [S, 64]
    sg = np.tile(np.array([-1.0, 1.0]), HALF)[None, :]
    sinn = (np.repeat(np.sin(fr), 2, axis=1) * sg).astype(np.float32)

    # [128, 128] lower-tri in (kv, q): valid iff q_off >= kv_off
    maskt = (np.arange(P)[None, :] >= np.arange(P)[:, None]).astype(bf)
    identity = np.eye(P, dtype=bf)
    cos_t = _tile_pm(cosn).astype(bf)                     # [128, 16*64]
    sin_t = _tile_pm(sinn).astype(bf)

    in_maps = []
    for c in range(N_CORES):
        b, hg = c // 2, c % 2
        rows = slice(hg * DOUT, (hg + 1) * DOUT)
        # x tiled: [p, m*1024 + k*128 + c] = x[b].T[k*128+p, m*128+c]
        # (partition = D dim: xs(m, k) is the matmul's stationary [d, s])
        xTb = np.ascontiguousarray(x[b].T)                # [1024, 2048]
        xt_t = xTb.reshape(KT, P, NSEQ, P).transpose(
            1, 2, 0, 3).reshape(P, -1).astype(bf)
        # weights: [p, k*512 + o] = w[d=k*128+p, o]  (w already transposed)
        wq_t = _tile_pm((wqf[rows] * SCALE).T).astype(bf)
        wk_t = _tile_pm(wkf[rows].T).astype(bf)
        wv_t = _tile_pm(wvf[rows].T).astype(bf)
        wo_t = _tile_pm(wof[:, rows].T).astype(bf)        # [128, 4*1024]
        in_maps.append({
            "xt": xt_t, "wq": wq_t, "wk": wk_t, "wv": wv_t, "wo": wo_t,
            "cosn": cos_t, "sinn": sin_t, "maskt": maskt, "ident": identity,
        })
    return in_maps


def kernel(q_proj_weight, k_proj_weight, v_proj_weight, o_proj_weight,
           in_features, token_positions):
    from concourse.bass_utils import run_bass_kernel_spmd

    nc = _get_nc()
    in_maps = _prep_core_inputs(q_proj_weight, k_proj_weight, v_proj_weight,
                                o_proj_weight, in_features, token_positions)
    trace = bool(int(os.environ.get("KBENCH_TRACE", "0")))
    res = run_bass_kernel_spmd(nc, in_maps, list(range(N_CORES)), trace=trace)
    _CACHE["last_results"] = res
    if res.exec_time_ns is not None:
        _CACHE["exec_time_ns"] = res.exec_time_ns

    outp = np.empty((B, S, D_MODEL), np.float32)
    for b in range(B):
        acc = None
        for hg in range(2):
            o = np.asarray(res.results[2 * b + hg]["out"], np.float32)
            o = o.reshape(P, NSEQ, D_MODEL).transpose(1, 0, 2).reshape(
                S, D_MODEL)
            acc = o if acc is None else acc + o
        outp[b] = acc
    return outp

